# revision 44
# baseline (speedup 1.0000x reference)
"""GCN (2-layer GCNConv) on 8 TRN2 NeuronCores via Bass/Tile.

Strategy (v2):
- Nodes sharded by dst across 8 cores (12544-row regions, 98 blocks of 128).
  Every edge lives on the core owning its dst. Two launches (one per layer);
  host glues the layer-1 output shards into the layer-2 input.
- Per layer, per core:
    Phase A: h = xT.T @ W (x pre-scaled by dinv host-side / by the previous
             layer's epilogue), written to 4 bucket-split DRAM tables so
             phase-B gathers of early buckets overlap the phase-A tail.
    Phase B: dma_gather rows h[src] (int16 bucket-local idx, small calls so
             desc-gen pipelines under transfers), one-hot matmul per 128-edge
             stripe accumulates into a PSUM-resident group of 8 dst blocks.
             Each block gets its OWN psum bank: the PE cannot interleave two
             open accumulation chains within one bank (only the last-stopped
             chain survives), while concurrently-open chains in different
             banks are fine. ACT epilogue applies the norm scale (+ReLU on
             layer 1).
- Cells keyed (group, bucket, block). Cell slot ranges share boundary stripes
  (no per-cell 128 roundup): a straddling stripe gets one matmul per cell,
  disambiguated by alternating dstl offsets (0/128) against a 256-wide iota.
- Node->(core, block) assignment balances per-cell edge counts across cores
  (greedy over per-bucket in-degree profiles); the shared SPMD layout pads
  each cell to its max-over-cores size, so balance is pure gather savings.
- norm = dinv[src]*dinv[dst] is separable: dinv[src] folded into the table,
  dinv[dst] applied in the epilogue (dinv^2 on layer 1 commutes with ReLU).
"""
import sys
sys.path.insert(0, "/opt/trn_rl_repo")
import numpy as np
import ml_dtypes
import concourse.bass as bass
import concourse.mybir as mybir
import concourse.tile as tile
from concourse import bacc
from concourse.bass_utils import run_bass_kernel_spmd

P = 128
GROUP = 8                 # dst blocks per psum-resident group (1 bank each)
CALL_CAP_STRIPES = 8      # max stripes per dma_gather call
A_CHUNK = 16              # node tiles per phase-A DMA chunk (2048 nodes)
NBLK = 98                 # blocks per core (12544 rows, 12500 used)
RPC_PAD = NBLK * P        # padded rows per core
# bucket boundaries over the padded table (multiples of A_CHUNK*P, <= 32768)
BSTART = [0, 26624, 53248, 79872, 100352]

bf16 = ml_dtypes.bfloat16


class Layout:
    """Shared (all-core) slot layout for one graph sharding."""
    pass


def build_layout(counts, ncores):
    """counts: [ncores, ncell] per-cell edge counts, cells ordered
    (group, bucket, block). Shared across cores (max)."""
    ngroup = (NBLK + GROUP - 1) // GROUP
    cells = []
    for g in range(ngroup):
        blocks = range(g * GROUP, min((g + 1) * GROUP, NBLK))
        for k in range(4):
            for b in blocks:
                cells.append((g, k, b))
    ncell = len(cells)
    assert counts.shape[1] == ncell
    maxc = counts.max(axis=0)

    cell_off = np.zeros(ncell, np.int64)
    parity = np.zeros(ncell, np.int64)
    calls = []                      # (slot_start, nslots, bucket)
    off = 0
    i = 0
    while i < ncell:
        # one run = consecutive cells with same (g, k)
        j = i
        while j + 1 < ncell and cells[j + 1][:2] == cells[i][:2]:
            j += 1
        run_start = off
        last_bstripe = -1
        par = 0
        for ci in range(i, j + 1):
            if off % P != 0 and off // P == last_bstripe:
                off = (off + P - 1) // P * P      # 3rd cell would share stripe
            if off % P != 0:
                last_bstripe = off // P
            cell_off[ci] = off
            parity[ci] = par
            par ^= 1
            off += maxc[ci]
        off = (off + P - 1) // P * P
        # split run into gather calls
        p0 = run_start
        while p0 < off:
            ns = min(off - p0, CALL_CAP_STRIPES * P)
            calls.append((p0, ns, cells[i][1]))
            p0 += ns
        i = j + 1

    L = Layout()
    L.cells = cells
    L.cell_off = cell_off
    L.cell_size = maxc.copy()
    L.parity = parity
    L.calls = calls
    L.nslot = int(off)
    L.nstripe = int(off) // P
    L.ngroup = ngroup

    # per-stripe list of (cell index) pieces; per-block first/last piece ids
    stripe_cells = [[] for _ in range(L.nstripe)]
    for ci in range(ncell):
        if maxc[ci] == 0:
            continue
        s0 = int(cell_off[ci]) // P
        s1 = (int(cell_off[ci]) + int(maxc[ci]) + P - 1) // P
        for s in range(s0, s1):
            stripe_cells[s].append(ci)
    L.stripe_cells = stripe_cells
    first_piece = {}
    last_piece = {}
    for s in range(L.nstripe):
        for ci in stripe_cells[s]:
            b = L.cells[ci][2]
            if b not in first_piece:
                first_piece[b] = (s, ci)
            last_piece[b] = (s, ci)
    L.first_piece = first_piece
    L.last_piece = last_piece
    return L


def host_prep(src_all, dst_all, N_nodes, ncores):
    """Shard + pack edges. Returns (layout, per-core data, dinv)."""
    deg = np.bincount(dst_all, minlength=N_nodes).astype(np.float64)
    dinv = 1.0 / np.sqrt(deg)

    # Free node->(core, block, row) assignment: snake-deal blocks by total
    # degree, then choose each node's core greedily to balance the per-cell
    # (bucket x block) edge counts across cores (the shared layout pads every
    # cell to its max-over-cores size, so imbalance is pure gather waste).
    order_n = np.argsort(-deg, kind="stable")
    rank = np.empty(N_nodes, np.int64)
    rank[order_n] = np.arange(N_nodes)
    nslots_deal = ncores * NBLK
    sweep = rank // nslots_deal
    pos = rank % nslots_deal
    pos = np.where(sweep % 2 == 1, nslots_deal - 1 - pos, pos)
    node_lblk = pos // ncores

    bucket_e = np.searchsorted(BSTART, src_all, side="right") - 1
    dprof = np.zeros((N_nodes, 4), np.int64)
    np.add.at(dprof, (dst_all, bucket_e), 1)
    node_core = np.zeros(N_nodes, np.int64)
    order_by_blk = np.lexsort((rank, node_lblk))
    blk_sorted = node_lblk[order_by_blk]
    for b in range(NBLK):
        lo, hi = np.searchsorted(blk_sorted, [b, b + 1])
        nodes = order_by_blk[lo:hi]
        cnt = np.zeros((ncores, 4))
        cap = np.zeros(ncores, np.int64)
        for n in nodes:
            prof = dprof[n]
            mx = cnt.max(axis=0)
            best, bestc = 1e18, -1
            for c in range(ncores):
                if cap[c] >= P:
                    continue
                v = np.maximum(cnt[c] + prof, mx).sum() + 1e-3 * cap[c]
                if v < best:
                    best, bestc = v, c
            node_core[n] = bestc
            cnt[bestc] += prof
            cap[bestc] += 1
    key = (node_core * NBLK + node_lblk)
    order2 = np.lexsort((rank, key))
    row_in_block = np.zeros(N_nodes, np.int64)
    kk = key[order2]
    starts = np.searchsorted(kk, np.arange(ncores * NBLK))
    row_in_block[order2] = np.arange(N_nodes) - np.repeat(
        starts, np.diff(np.append(starts, N_nodes)))
    assert row_in_block.max() < P
    node_lrow = node_lblk * P + row_in_block
    # h table stays in global node-id order: self-loops and src mass then
    # spread uniformly over the (2048-aligned) id-range buckets, keeping
    # cross-core cell sizes balanced.
    table_row = np.arange(N_nodes, dtype=np.int64)

    bstart = np.asarray(BSTART[:-1], np.int64)
    ngroup = (NBLK + GROUP - 1) // GROUP
    cells = []
    cell_rank = np.full((4, NBLK), -1, np.int64)
    for g in range(ngroup):
        for k in range(4):
            for b in range(g * GROUP, min((g + 1) * GROUP, NBLK)):
                cell_rank[k, b] = len(cells)
                cells.append((g, k, b))
    ncell = len(cells)

    ecore = node_core[dst_all]
    core_edges = []
    counts = np.zeros((ncores, ncell), np.int64)
    for c in range(ncores):
        m = ecore == c
        s_r = table_row[src_all[m]]
        d_l = node_lrow[dst_all[m]]
        bucket = np.searchsorted(BSTART, s_r, side="right") - 1
        block = d_l >> 7
        cr = cell_rank[bucket, block]
        counts[c] = np.bincount(cr, minlength=ncell)
        core_edges.append((s_r, d_l, bucket, cr))

    L = build_layout(counts, ncores)

    cores = []
    for c in range(ncores):
        s_r, d_l, bucket, cr = core_edges[c]
        order = np.lexsort((s_r, cr))
        s_r, d_l, bucket, cr = s_r[order], d_l[order], bucket[order], cr[order]
        uniq, start, cnt = np.unique(cr, return_index=True, return_counts=True)
        within = np.arange(len(s_r)) - np.repeat(start, cnt)
        slot = L.cell_off[cr] + within

        nslot = L.nslot
        idx_local = np.zeros(nslot, np.int16)
        dstl = np.full(nslot, -1.0, np.float32)
        idx_local[slot] = (s_r - bstart[bucket]).astype(np.int16)
        dstl[slot] = ((d_l & 127) + P * L.parity[cr]).astype(np.float32)

        idx_arr = np.zeros((16, nslot // 16), np.int16)
        idx_arr[np.arange(nslot) % 16, np.arange(nslot) // 16] = idx_local
        idx_arr = np.tile(idx_arr, (8, 1))

        dstl_arr = np.zeros((P, L.nstripe), np.float32)
        dstl_arr[np.arange(nslot) % P, np.arange(nslot) // P] = dstl

        mine = np.where(node_core == c)[0]
        deg_c = np.zeros(RPC_PAD, np.float64)
        dinv_c = np.zeros(RPC_PAD, np.float64)
        rowmap = np.full(RPC_PAD, -1, np.int64)
        deg_c[node_lrow[mine]] = deg[mine]
        dinv_c[node_lrow[mine]] = dinv[mine]
        rowmap[node_lrow[mine]] = mine

        cores.append(dict(
            idx_arr=idx_arr, dstl_arr=dstl_arr,
            dinv=dinv_c.astype(np.float32),
            sqd=np.sqrt(deg_c).astype(np.float32),
            rowmap=rowmap, table_row=table_row,
        ))
    return L, cores, dinv


def build_layer(N_nodes, L, relu, out_cols, out_dtype, use_bias):
    """Build one GCN layer program (SPMD, shared across cores)."""
    NPAD = BSTART[-1]
    NT = NPAD // (P * A_CHUNK)
    nstripe = L.nstripe
    nslot = L.nslot

    nc = bacc.Bacc("TRN2", target_bir_lowering=False, debug=True)
    xT = nc.declare_dram_parameter("xT", [P, NPAD], mybir.dt.bfloat16, isOutput=False)
    W = nc.declare_dram_parameter("W", [P, P], mybir.dt.bfloat16, isOutput=False)
    brow = nc.declare_dram_parameter("brow", [1, NBLK * P + P], mybir.dt.bfloat16, isOutput=False)
    cst = nc.declare_dram_parameter("cst", [P, NBLK + nstripe], mybir.dt.float32, isOutput=False)
    cstb = nc.declare_dram_parameter("cstb", [P, 2 * P], mybir.dt.bfloat16, isOutput=False)
    idx = nc.declare_dram_parameter("idx", [P, nslot // 16], mybir.dt.int16, isOutput=False)
    out = nc.declare_dram_parameter("out", [P, NBLK * out_cols], out_dtype, isOutput=True)
    hbuf = [nc.dram_tensor(f"h{k}", [BSTART[k + 1] - BSTART[k], P], mybir.dt.bfloat16)
            for k in range(4)]

    with tile.TileContext(nc) as tc:
        with (
            tc.tile_pool(name="const", bufs=1) as cpool,
            tc.tile_pool(name="xin", bufs=4) as xpool,
            tc.tile_pool(name="hout", bufs=4) as hpool,
            tc.tile_pool(name="msg", bufs=6) as mpool,
            tc.tile_pool(name="sbuild", bufs=16) as spool,
            tc.tile_pool(name="oeps", bufs=6) as opool,
        ):
            W_t = cpool.tile([P, P], mybir.dt.bfloat16)
            nc.sync.dma_start(out=W_t[:], in_=W[:])
            brow_t = cpool.tile([1, NBLK * P + P], mybir.dt.bfloat16)
            cst_t = cpool.tile([P, NBLK + nstripe], mybir.dt.float32)
            cstb_t = cpool.tile([P, 2 * P], mybir.dt.bfloat16)
            idx_t = cpool.tile([P, nslot // 16], mybir.dt.int16)

            scl_t = cst_t[:, 0:NBLK]
            dstl_t = cst_t[:, NBLK:]
            iota_t = cstb_t                      # [P, 256] iota
            sqd_t = brow_t[:, 0:NBLK * P]
            brhs_t = brow_t[:, NBLK * P:]

            # ---- Phase A: h = x @ W ----
            psA_ctx = tc.tile_pool(name="psA", bufs=8, space="PSUM")
            psA = psA_ctx.__enter__()
            # phase-B constants on the ACT DMA queue ahead of phase A
            nc.scalar.dma_start(out=brow_t[:], in_=brow[:])
            nc.scalar.dma_start(out=cst_t[:], in_=cst[:])
            nc.scalar.dma_start(out=cstb_t[:], in_=cstb[:])
            for c in range(NT):
                xt = xpool.tile([P, A_CHUNK * P], mybir.dt.bfloat16, tag="xt")
                nc.sync.dma_start(out=xt[:], in_=xT[:, c * A_CHUNK * P:(c + 1) * A_CHUNK * P])
                hb = hpool.tile([P, A_CHUNK, P], mybir.dt.bfloat16, tag="hb")
                for half in range(A_CHUNK // 4):
                    ps = psA.tile([P, 4 * P], mybir.dt.float32, space="PSUM", tag="psA")
                    for j in range(4):
                        sj = half * 4 + j
                        nc.tensor.matmul(
                            out=ps[:, j * P:(j + 1) * P],
                            lhsT=xt[:, sj * P:(sj + 1) * P],
                            rhs=W_t[:], start=True, stop=True)
                    nc.scalar.activation(
                        out=hb[:, half * 4:(half + 1) * 4, :].rearrange("p s f -> p (s f)"),
                        in_=ps[:], func=mybir.ActivationFunctionType.Copy)
                r0 = c * A_CHUNK * P
                bk = next(k for k in range(4) if BSTART[k] <= r0 < BSTART[k + 1])
                rloc = r0 - BSTART[bk]
                nc.scalar.dma_start(
                    out=hbuf[bk][rloc:rloc + A_CHUNK * P, :].rearrange(
                        "(p s) f -> p (s f)", p=P),
                    in_=hb[:, :, :].rearrange("p s f -> p (s f)"))
            psA_ctx.__exit__(None, None, None)

            # ---- Phase B: psum-resident groups of GROUP dst blocks ----
            psB_ctx = tc.tile_pool(name="psB", bufs=8, space="PSUM")
            psB = psB_ctx.__enter__()

            cells = L.cells
            started = set()
            # map stripe -> (call id, offset within call)
            call_of_stripe = {}
            for cid, (s0, ns, bk) in enumerate(L.calls):
                for t in range(ns // P):
                    call_of_stripe[s0 // P + t] = (cid, t)

            call_tiles = {}
            emitted = set()

            def ensure_call(cid):
                if cid in emitted:
                    return
                emitted.add(cid)
                s0, ns, bk = L.calls[cid]
                mt = mpool.tile([P, CALL_CAP_STRIPES, P], mybir.dt.bfloat16,
                                tag="msg", name=f"msg{cid}")
                nc.gpsimd.dma_gather(
                    out_ap=mt[:, :ns // P, :],
                    in_ap=hbuf[bk][:, :],
                    idxs_ap=idx_t[:, s0 // 16:(s0 + ns) // 16],
                    num_idxs=ns, num_idxs_reg=ns, elem_size=P,
                    single_packet=False)
                call_tiles[cid] = mt

            nstr_emitted = 0
            for g in range(L.ngroup):
                g0 = g * GROUP
                gblocks = list(range(g0, min(g0 + GROUP, NBLK)))
                pb = {b: psB.tile([P, P], mybir.dt.float32, space="PSUM",
                                  tag="psB", name=f"pb{b}")
                      for b in gblocks}
                # stripe range of group g: from first cell of (g,0) to end of
                # last cell of (g,3)
                ci_first = cells.index((g, 0, gblocks[0]))
                ci_last = cells.index((g, 3, gblocks[-1]))
                s_begin = int(L.cell_off[ci_first]) // P
                s_end = (int(L.cell_off[ci_last]) + int(L.cell_size[ci_last]) + P - 1) // P
                # just-in-time idx slice for this group's gather calls: keeps
                # the bulk of the idx traffic off the phase-A critical path
                g0c = s_begin * P // 16
                g1c = s_end * P // 16
                nc.scalar.dma_start(out=idx_t[:, g0c:g1c], in_=idx[:, g0c:g1c])
                for s in range(s_begin, s_end):
                    if not L.stripe_cells[s]:
                        continue
                    cid, t = call_of_stripe[s]
                    ensure_call(cid)
                    mt = call_tiles[cid]
                    for ci in L.stripe_cells[s]:
                        gg, kk_, b = cells[ci]
                        assert gg == g
                        par = int(L.parity[ci])
                        S = spool.tile([P, P], mybir.dt.bfloat16, tag="S",
                                       name=f"S{nstr_emitted}")
                        nstr_emitted += 1
                        nc.vector.tensor_scalar(
                            out=S[:], in0=iota_t[:, par * P:(par + 1) * P],
                            scalar1=dstl_t[:, s:s + 1], scalar2=None,
                            op0=mybir.AluOpType.is_equal)
                        first = b not in started
                        if first and use_bias:
                            nc.tensor.matmul(
                                out=pb[b][:],
                                lhsT=sqd_t[:, b * P:(b + 1) * P],
                                rhs=brhs_t[:], start=True, stop=False)
                            first = False
                        started.add(b)
                        is_last = (L.last_piece[b] == (s, ci))
                        nc.tensor.matmul(
                            out=pb[b][:], lhsT=S[:], rhs=mt[:, t, :],
                            start=first, stop=is_last)
                # epilogue: quads of blocks share one output tile so each
                # partition writes a contiguous >=512B run (full-rate DMA)
                for q0 in range(0, len(gblocks), 4):
                    qb = gblocks[q0:q0 + 4]
                    ot = opool.tile([P, 4, out_cols], out_dtype, tag="ot",
                                    name=f"ot{qb[0]}")
                    for j, b in enumerate(qb):
                        nc.scalar.activation(
                            out=ot[:, j, :], in_=pb[b][:, :out_cols],
                            func=(mybir.ActivationFunctionType.Relu if relu
                                  else mybir.ActivationFunctionType.Copy),
                            scale=scl_t[:, b:b + 1])
                    nc.sync.dma_start(
                        out=out[:, qb[0] * out_cols:(qb[0] + len(qb)) * out_cols],
                        in_=ot[:, :len(qb), :].rearrange("p s f -> p (s f)"))
            psB_ctx.__exit__(None, None, None)
    nc.compile()
    return nc


def make_layer_inputs(L, cores, xT_pad, Wp, bp, scl_per_core):
    in_maps = []
    nstripe = L.nstripe
    for c, core in enumerate(cores):
        brow = np.zeros((1, NBLK * P + P), bf16)
        brow[0, :NBLK * P] = core["sqd"].astype(bf16)
        brow[0, NBLK * P:] = bp.astype(bf16)
        cst = np.zeros((P, NBLK + nstripe), np.float32)
        cst[:, :NBLK] = scl_per_core[c].reshape(NBLK, P).T
        cst[:, NBLK:] = core["dstl_arr"]
        cstb = np.tile(np.arange(2 * P, dtype=np.float32), (P, 1)).astype(bf16)
        in_maps.append({
            "xT": xT_pad, "W": Wp, "brow": brow, "cst": cst, "cstb": cstb,
            "idx": core["idx_arr"],
        })
    return in_maps


def permute_chunks(xT):
    """Within each A_CHUNK*128-col chunk, permute cols so phase-A h writes are
    4KB-contiguous per partition."""
    Pn, npad = xT.shape
    nch = npad // (P * A_CHUNK)
    v = xT.reshape(Pn, nch, P, A_CHUNK)
    return np.ascontiguousarray(v.transpose(0, 1, 3, 2)).reshape(Pn, npad)


_prep_cache = {}


def gcn_kernel(edge_index, node_emb, W1, b1, W2, b2, ncores=8, verbose=False,
               trace=False):
    import time
    N_nodes, EMB = node_emb.shape
    REPR = W2.shape[1]

    src_all = np.concatenate([np.asarray(edge_index[0]), np.arange(N_nodes)]).astype(np.int64)
    dst_all = np.concatenate([np.asarray(edge_index[1]), np.arange(N_nodes)]).astype(np.int64)

    t0 = time.time()
    ckey = (hash(src_all.tobytes()) ^ hash(dst_all.tobytes()), N_nodes, ncores)
    if ckey in _prep_cache:
        L, cores, dinv = _prep_cache[ckey]["prep"]
    else:
        _prep_cache.clear()
        _prep_cache[ckey] = {}
        _prep_cache[ckey]["prep"] = host_prep(src_all, dst_all, N_nodes, ncores)
        L, cores, dinv = _prep_cache[ckey]["prep"]
    if verbose:
        real = len(src_all)
        print(f"host_prep: {time.time()-t0:.2f}s nslot={L.nslot*ncores} "
              f"(pad {(L.nslot*ncores - real)/real:.1%}) calls={len(L.calls)}",
              flush=True)

    NPAD = BSTART[-1]
    table_row = cores[0]["table_row"]

    results = {}
    # ---- layer 1 ----
    x1 = (dinv[:, None] * np.asarray(node_emb, np.float64)).astype(bf16)
    xT1 = np.zeros((P, NPAD), bf16)
    xT1[:, table_row] = x1.T.astype(bf16)[:, :]
    xT1 = permute_chunks(xT1)
    W1p = np.asarray(W1, np.float32).astype(bf16)
    scl1 = [c["dinv"] ** 2 for c in cores]

    t0 = time.time()
    cache = _prep_cache[ckey]
    k1 = ("L1", bool(np.any(np.asarray(b1))))
    if k1 not in cache:
        cache[k1] = build_layer(N_nodes, L, relu=True, out_cols=P,
                                out_dtype=mybir.dt.bfloat16, use_bias=k1[1])
    nc1 = cache[k1]
    if verbose:
        print(f"build L1: {time.time()-t0:.2f}s", flush=True)
    in1 = make_layer_inputs(L, cores, xT1, W1p, np.asarray(b1, np.float32), scl1)
    t0 = time.time()
    res1 = run_bass_kernel_spmd(nc1, in1, list(range(ncores)), trace=trace)
    results["L1"] = res1
    if verbose:
        print(f"run L1: {time.time()-t0:.2f}s exec_ns={res1.exec_time_ns}", flush=True)

    # glue: scatter layer-1 output shards back to global node-id order
    # (device output is partition-major: out[p, b*128+f] = row b*128+p)
    xT2 = np.zeros((P, NPAD), bf16)
    for c in range(ncores):
        rm = cores[c]["rowmap"]
        v = rm >= 0
        arr = res1.results[c]["out"].reshape(P, NBLK, P)
        rows = np.ascontiguousarray(arr.transpose(1, 0, 2)).reshape(RPC_PAD, P)
        xT2[:, rm[v]] = rows[v].T
    xT2 = permute_chunks(xT2)
    W2p = np.zeros((P, P), bf16)
    W2p[:, :REPR] = np.asarray(W2, np.float32).astype(bf16)
    scl2 = [c["dinv"] for c in cores]
    b2p = np.zeros(P, np.float32)
    b2p[:REPR] = np.asarray(b2, np.float32)

    t0 = time.time()
    k2 = ("L2", REPR, bool(np.any(b2p)))
    if k2 not in cache:
        cache[k2] = build_layer(N_nodes, L, relu=False, out_cols=REPR,
                                out_dtype=mybir.dt.float32, use_bias=k2[2])
    nc2 = cache[k2]
    if verbose:
        print(f"build L2: {time.time()-t0:.2f}s", flush=True)
    in2 = make_layer_inputs(L, cores, xT2, W2p, b2p, scl2)
    t0 = time.time()
    res2 = run_bass_kernel_spmd(nc2, in2, list(range(ncores)), trace=trace)
    results["L2"] = res2
    if verbose:
        print(f"run L2: {time.time()-t0:.2f}s exec_ns={res2.exec_time_ns}", flush=True)

    out = np.zeros((N_nodes, REPR), np.float32)
    for c in range(ncores):
        rm = cores[c]["rowmap"]
        v = rm >= 0
        arr = res2.results[c]["out"].reshape(P, NBLK, REPR)
        rows = np.ascontiguousarray(arr.transpose(1, 0, 2)).reshape(RPC_PAD, REPR)
        out[rm[v]] = rows[v]
    return out, results


def kernel(edge_index, node_emb, W1, b1, W2, b2):
    """Self-contained entry point: full inputs -> full output [N, REPR] f32."""
    out, _ = gcn_kernel(np.asarray(edge_index), np.asarray(node_emb),
                        np.asarray(W1), np.asarray(b1),
                        np.asarray(W2), np.asarray(b2), ncores=8)
    return out
